# revision 6
# baseline (speedup 1.0000x reference)
import numpy as np

N_BATCH = 2
T = 2048
D = 1024
HG = 4
NH_LOC = 4
DK = 64
HD = NH_LOC * DK
P = 128
TQ = 512
NTC = T // P
NQT = T // TQ
DC = D // P

_NC = None


def _build(loop_iters=None, trace_sim=False):
    import contextlib
    import concourse.bass as bass
    from concourse import bacc
    import concourse.mybir as mybir
    import concourse.tile as tile
    from concourse.masks import make_identity

    F32 = mybir.dt.float32
    F32R = mybir.dt.float32r
    BF16 = mybir.dt.bfloat16
    I32 = mybir.dt.int32
    Exp = mybir.ActivationFunctionType.Exp
    Ident = mybir.ActivationFunctionType.Identity
    mult = mybir.AluOpType.mult

    nc = bacc.Bacc(None, target_bir_lowering=False)
    xq = nc.dram_tensor("xq", [T, D], F32, kind="ExternalInput")
    xk = nc.dram_tensor("xk", [T, D], F32, kind="ExternalInput")
    xv = nc.dram_tensor("xv", [T, D], F32, kind="ExternalInput")
    mask = nc.dram_tensor("mask", [T], I32, kind="ExternalInput")
    wq = nc.dram_tensor("wq", [D, HD], F32, kind="ExternalInput")
    wk = nc.dram_tensor("wk", [D, HD], F32, kind="ExternalInput")
    wv = nc.dram_tensor("wv", [D, HD], F32, kind="ExternalInput")
    bq = nc.dram_tensor("bq", [HD], F32, kind="ExternalInput")
    bk = nc.dram_tensor("bk", [HD], F32, kind="ExternalInput")
    bv = nc.dram_tensor("bv", [HD], F32, kind="ExternalInput")
    wo = nc.dram_tensor("wo", [HD, D], F32, kind="ExternalInput")
    out = nc.dram_tensor("out", [T, D], F32, kind="ExternalOutput")

    with tile.TileContext(nc, trace_sim=trace_sim) as tc:
        loop_cm = tc.For_i(0, loop_iters, 1) if loop_iters else contextlib.nullcontext()
        with loop_cm, \
             tc.tile_pool(name="const", bufs=1) as const, \
             tc.tile_pool(name="stage", bufs=3) as stage, \
             tc.tile_pool(name="xt", bufs=2) as xtp, \
             tc.tile_pool(name="kv", bufs=1) as kvp, \
             tc.tile_pool(name="qz", bufs=5) as qzp, \
             tc.tile_pool(name="et", bufs=3) as etp, \
             tc.tile_pool(name="sm", bufs=4) as smp, \
             tc.tile_pool(name="ot", bufs=2) as otp, \
             tc.tile_pool(name="tp", bufs=1, space="PSUM") as tpp, \
             tc.tile_pool(name="pp", bufs=2, space="PSUM") as ppp, \
             tc.tile_pool(name="sc", bufs=2, space="PSUM") as scp, \
             tc.tile_pool(name="pa", bufs=1, space="PSUM") as pap:

            ident = const.tile([P, P], BF16, name="ident")
            make_identity(nc, ident)
            shiftI = const.tile([DK, P], BF16, name="shiftI")
            id64 = const.tile([DK, DK], BF16, name="id64")
            make_identity(nc, id64)
            nc.vector.memset(shiftI[:, 0:DK], 0.0)
            nc.vector.tensor_copy(shiftI[:, DK:P], id64[:])

            wq_sb = const.tile([P, DC, HD], BF16, name="wq_sb")
            wk_sb = const.tile([P, DC, HD], BF16, name="wk_sb")
            wv_sb = const.tile([P, DC, HD], BF16, name="wv_sb")
            nc.gpsimd.dma_start(wq_sb[:], wq.rearrange("(dc p) h -> p dc h", p=P))
            nc.gpsimd.dma_start(wk_sb[:], wk.rearrange("(dc p) h -> p dc h", p=P))
            nc.gpsimd.dma_start(wv_sb[:], wv.rearrange("(dc p) h -> p dc h", p=P))
            wo_sb = const.tile([P, 2, D], BF16, name="wo_sb")
            nc.gpsimd.dma_start(wo_sb[:], wo.rearrange("(hp p) n -> p hp n", p=P))

            bq_sb = const.tile([P, 2], F32, name="bq_sb")
            bk_sb = const.tile([P, 2], F32, name="bk_sb")
            nc.sync.dma_start(bq_sb[:], bq.rearrange("(hc p) -> p hc", p=P))
            nc.sync.dma_start(bk_sb[:], bk.rearrange("(hc p) -> p hc", p=P))
            bv_row = const.tile([1, HD], BF16, name="bv_row")
            nc.gpsimd.dma_start(bv_row[:], bv[None, :])
            ones_row = const.tile([1, P], BF16, name="ones_row")
            nc.vector.memset(ones_row[:], 1.0)
            onesr_f = const.tile([65, DK], F32, name="onesr_f")
            nc.vector.memset(onesr_f[64:65, :], 1.0)
            onesr = const.tile([65, DK], F32R, name="onesr")
            nc.vector.tensor_copy(onesr[64:65, :], onesr_f[64:65, :])

            m_i32 = const.tile([P, NTC], I32, name="m_i32")
            nc.sync.dma_start(m_i32[:], mask.rearrange("(o p) -> p o", p=P))
            m_f32 = const.tile([P, NTC], F32, name="m_f32")
            nc.vector.tensor_copy(m_f32[:], m_i32[:])

            khT = [kvp.tile([P, 2, TQ], BF16, name=f"khT{j}") for j in range(NQT)]
            vh = [kvp.tile([P, 4, NH_LOC * 65], BF16, name=f"vh{j}")
                  for j in range(NQT)]
            for j in range(NQT):
                for h in range(NH_LOC):
                    nc.vector.tensor_copy(vh[j][:, :, h * 65 + 64],
                                          m_f32[:, 4 * j:4 * j + 4])

            def load_transposed(src, tag):
                for qt in range(NQT):
                    xtc = xtp.tile([P, DC, TQ], BF16, name=f"xt_{tag}{qt}",
                                   tag="xt")
                    for t4 in range(4):
                        tci = qt * 4 + t4
                        nat = stage.tile([P, D], BF16, name=f"nat_{tag}{tci}",
                                         tag="nat")
                        nc.gpsimd.dma_start(nat[:], src[tci * P:(tci + 1) * P, :])
                        ptp = tpp.tile([P, DC, P], BF16, name=f"tp_{tag}{tci}",
                                       tag="tp")
                        for dc in range(DC):
                            nc.tensor.transpose(
                                ptp[:, dc, :], nat[:, dc * P:(dc + 1) * P], ident[:])
                        nc.vector.tensor_copy(
                            xtc[:, :, t4 * P:(t4 + 1) * P], ptp[:])
                    yield qt, xtc

            kgen = load_transposed(xk, "k")
            qgen = load_transposed(xq, "q")
            qzs = []
            for qt in range(NQT):
                _, ktc = next(kgen)
                for hc in range(2):
                    pp = ppp.tile([P, TQ], F32, name=f"ppk{qt}_{hc}", tag="pp")
                    for dc in range(DC):
                        nc.tensor.matmul(pp[:], wk_sb[:, dc, hc * P:(hc + 1) * P],
                                         ktc[:, dc, :],
                                         start=(dc == 0), stop=(dc == DC - 1))
                    nc.vector.tensor_scalar_add(khT[qt][:, hc, :], pp[:],
                                                bk_sb[:, hc:hc + 1])
                _, qtc = next(qgen)
                qz = qzp.tile([P, NH_LOC, TQ], BF16, name=f"qz{qt}", tag="qz")
                for hc in range(2):
                    pp = ppp.tile([P, TQ], F32, name=f"ppq{qt}_{hc}", tag="pp")
                    for dc in range(DC):
                        nc.tensor.matmul(pp[:], wq_sb[:, dc, hc * P:(hc + 1) * P],
                                         qtc[:, dc, :],
                                         start=(dc == 0), stop=(dc == DC - 1))
                    nc.vector.tensor_scalar_add(qz[0:DK, 2 * hc, :], pp[0:DK, :],
                                                bq_sb[0:DK, hc:hc + 1])
                    nc.vector.tensor_scalar_add(qz[DK:P, 2 * hc + 1, :], pp[DK:P, :],
                                                bq_sb[DK:P, hc:hc + 1])
                    nc.vector.memset(qz[DK:P, 2 * hc, :], 0.0)
                    nc.vector.memset(qz[0:DK, 2 * hc + 1, :], 0.0)
                qzs.append(qz)

            for qt, vtc in load_transposed(xv, "v"):
                for t4 in range(4):
                    pp = ppp.tile([P, TQ], F32, name=f"ppv{qt}_{t4}", tag="pp")
                    for dc in range(DC):
                        nc.tensor.matmul(pp[:, 0:HD], vtc[:, dc, t4 * P:(t4 + 1) * P],
                                         wv_sb[:, dc, :],
                                         start=(dc == 0), stop=False)
                    nc.tensor.matmul(pp[:, 0:HD], ones_row[:], bv_row[:],
                                     start=False, stop=True)
                    tci = qt * 4 + t4
                    nc.vector.tensor_scalar_mul(
                        vh[qt][:, t4, :].rearrange("p (h x) -> p h x",
                                                   x=65)[:, :, 0:DK],
                        pp[:, 0:HD].rearrange("p (h x) -> p h x", x=DK),
                        m_f32[:, tci:tci + 1])

            for qt in range(NQT):
                qz = qzs[qt]
                at_pairs = []
                for hp in range(2):
                    at_pair = smp.tile([P, TQ], BF16, name=f"atp{qt}_{hp}",
                                       tag="at")
                    for hi in range(2):
                        h = 2 * hp + hi
                        eh = etp.tile([P, NTC, TQ], BF16, name=f"e{qt}_{h}",
                                      tag="e")
                        for kc2 in range(NTC // 2):
                            ps = scp.tile([P, 2, TQ], F32, name=f"s{qt}_{h}_{kc2}",
                                          tag="s")
                            for j in range(2):
                                kc = 2 * kc2 + j
                                nc.tensor.matmul(
                                    ps[:, j, :],
                                    khT[kc // 4][:, hp, (kc % 4) * P:(kc % 4 + 1) * P],
                                    qz[:, h, :], start=True, stop=True)
                            nc.scalar.activation(eh[:, 2 * kc2:2 * kc2 + 2, :],
                                                 ps[:], Exp, scale=0.125)
                        pa = pap.tile([65, TQ], F32, name=f"pa{qt}_{h}", tag="pa")
                        for kc in range(NTC):
                            nc.tensor.matmul(
                                pa[:], vh[kc // 4][:, kc % 4, h * 65:h * 65 + 65],
                                eh[:, kc, :],
                                start=(kc == 0), stop=(kc == NTC - 1))
                        rec = smp.tile([65, TQ], F32R, name=f"rec{qt}_{h}",
                                       tag="rec")
                        with nc.allow_low_precision(reason="softmax denominator"):
                            nc.vector.reciprocal(rec[64:65, :], pa[64:65, :])
                        pb = scp.tile([P, 2, TQ], F32, name=f"pb{qt}_{h}", tag="s")
                        nc.tensor.matmul(pb[0:DK, 0, :], onesr[64:65, :],
                                         rec[64:65, :], start=True, stop=True)
                        au = smp.tile([DK, TQ], BF16, name=f"au{qt}_{h}", tag="au")
                        nc.vector.tensor_copy(au[:], pa[0:DK, :])
                        if hi == 0:
                            nc.vector.tensor_tensor(at_pair[0:DK, :], au[:],
                                                    pb[0:DK, 0, :], mult)
                        else:
                            att = smp.tile([DK, TQ], BF16, name=f"att{qt}_{h}",
                                           tag="att")
                            nc.vector.tensor_tensor(att[:], au[:],
                                                    pb[0:DK, 0, :], mult)
                            psh = scp.tile([P, 2, TQ], F32, name=f"sh{qt}_{h}",
                                           tag="s")
                            nc.tensor.matmul(psh[:, 0, :], shiftI[:], att[:],
                                             start=True, stop=True)
                            nc.vector.tensor_copy(at_pair[DK:P, :],
                                                  psh[DK:P, 0, :])
                    at_pairs.append(at_pair)

                for t4 in range(4):
                    osb = otp.tile([P, D], F32, name=f"o{qt}_{t4}", tag="o")
                    for nh in range(2):
                        po = ppp.tile([P, TQ], F32, name=f"po{qt}_{t4}_{nh}",
                                      tag="pp")
                        for hp in range(2):
                            nc.tensor.matmul(
                                po[:], at_pairs[hp][:, t4 * P:(t4 + 1) * P],
                                wo_sb[:, hp, nh * TQ:(nh + 1) * TQ],
                                start=(hp == 0), stop=(hp == 1))
                        nc.vector.tensor_copy(osb[:, nh * TQ:(nh + 1) * TQ], po[:])
                    tci = qt * 4 + t4
                    nc.sync.dma_start(out[tci * P:(tci + 1) * P, :], osb[:])

    nc.compile()
    return nc


def _get_nc():
    global _NC
    if _NC is None:
        _NC = _build()
    return _NC


def kernel(q, k, v, mask, Wq, bq, Wk, bk, Wv, bv, Wo, bo):
    from concourse.bass_utils import run_bass_kernel_spmd

    nc = _get_nc()
    c = np.ascontiguousarray
    in_maps = []
    for core in range(8):
        b, hg = divmod(core, HG)
        s = slice(hg * HD, (hg + 1) * HD)
        in_maps.append({
            "xq": c(np.asarray(q[b], dtype=np.float32)),
            "xk": c(np.asarray(k[b], dtype=np.float32)),
            "xv": c(np.asarray(v[b], dtype=np.float32)),
            "mask": c(np.asarray(mask[b], dtype=np.int32)),
            "wq": c(np.asarray(Wq[:, s], dtype=np.float32)),
            "wk": c(np.asarray(Wk[:, s], dtype=np.float32)),
            "wv": c(np.asarray(Wv[:, s], dtype=np.float32)),
            "bq": c(np.asarray(bq[s], dtype=np.float32)),
            "bk": c(np.asarray(bk[s], dtype=np.float32)),
            "bv": c(np.asarray(bv[s], dtype=np.float32)),
            "wo": c(np.asarray(Wo[s, :], dtype=np.float32)),
        })
    res = run_bass_kernel_spmd(nc, in_maps, list(range(8)))
    outs = np.empty((N_BATCH, T, D), dtype=np.float32)
    for b in range(N_BATCH):
        acc = res.results[b * HG]["out"].astype(np.float32).copy()
        for hg in range(1, HG):
            acc += res.results[b * HG + hg]["out"]
        outs[b] = acc + np.asarray(bo, dtype=np.float32)[None, :]
    return outs


# revision 7
# speedup vs baseline: 1.2436x; 1.2436x over previous
import numpy as np

N_BATCH = 2
T = 2048
D = 1024
HG = 4
NH_LOC = 4
DK = 64
HD = NH_LOC * DK
P = 128
TQ = 512
NTC = T // P
NQT = T // TQ
DC = D // P

_NC = None


def _build(loop_iters=None, trace_sim=False):
    import contextlib
    import concourse.bass as bass
    from concourse import bacc
    import concourse.mybir as mybir
    import concourse.tile as tile
    from concourse.masks import make_identity

    F32 = mybir.dt.float32
    F32R = mybir.dt.float32r
    BF16 = mybir.dt.bfloat16
    I32 = mybir.dt.int32
    Exp = mybir.ActivationFunctionType.Exp
    Ident = mybir.ActivationFunctionType.Identity
    mult = mybir.AluOpType.mult

    nc = bacc.Bacc(None, target_bir_lowering=False)
    xq = nc.dram_tensor("xq", [T, D], F32, kind="ExternalInput")
    xk = nc.dram_tensor("xk", [T, D], F32, kind="ExternalInput")
    xv = nc.dram_tensor("xv", [T, D], F32, kind="ExternalInput")
    mask = nc.dram_tensor("mask", [T], I32, kind="ExternalInput")
    wq = nc.dram_tensor("wq", [D, HD], F32, kind="ExternalInput")
    wk = nc.dram_tensor("wk", [D, HD], F32, kind="ExternalInput")
    wv = nc.dram_tensor("wv", [D, HD], F32, kind="ExternalInput")
    bq = nc.dram_tensor("bq", [HD], F32, kind="ExternalInput")
    bk = nc.dram_tensor("bk", [HD], F32, kind="ExternalInput")
    bv = nc.dram_tensor("bv", [HD], F32, kind="ExternalInput")
    wo = nc.dram_tensor("wo", [HD, D], F32, kind="ExternalInput")
    out = nc.dram_tensor("out", [T, D], BF16, kind="ExternalOutput")

    with tile.TileContext(nc, trace_sim=trace_sim) as tc:
        loop_cm = tc.For_i(0, loop_iters, 1) if loop_iters else contextlib.nullcontext()
        with loop_cm, \
             tc.tile_pool(name="const", bufs=1) as const, \
             tc.tile_pool(name="stage", bufs=6) as stage, \
             tc.tile_pool(name="xt", bufs=2) as xtp, \
             tc.tile_pool(name="kv", bufs=1) as kvp, \
             tc.tile_pool(name="qz", bufs=2) as qzp, \
             tc.tile_pool(name="et", bufs=3) as etp, \
             tc.tile_pool(name="sm", bufs=4) as smp, \
             tc.tile_pool(name="ot", bufs=2) as otp, \
             tc.tile_pool(name="tp", bufs=1, space="PSUM") as tpp, \
             tc.tile_pool(name="pp", bufs=2, space="PSUM") as ppp, \
             tc.tile_pool(name="sc", bufs=2, space="PSUM") as scp, \
             tc.tile_pool(name="pa", bufs=1, space="PSUM") as pap:

            ident = const.tile([P, P], BF16, name="ident")
            make_identity(nc, ident)
            shiftI = const.tile([DK, P], BF16, name="shiftI")
            id64 = const.tile([DK, DK], BF16, name="id64")
            make_identity(nc, id64)
            nc.vector.memset(shiftI[:, 0:DK], 0.0)
            nc.vector.tensor_copy(shiftI[:, DK:P], id64[:])

            wq_sb = const.tile([P, DC, HD], BF16, name="wq_sb")
            wk_sb = const.tile([P, DC, HD], BF16, name="wk_sb")
            wv_sb = const.tile([P, DC, HD], BF16, name="wv_sb")
            nc.gpsimd.dma_start(wk_sb[:], wk.rearrange("(dc p) h -> p dc h", p=P))
            wo_sb = const.tile([P, 2, D], BF16, name="wo_sb")

            bq_sb = const.tile([P, 2], F32, name="bq_sb")
            bk_sb = const.tile([P, 2], F32, name="bk_sb")
            nc.sync.dma_start(bq_sb[:], bq.rearrange("(hc p) -> p hc", p=P))
            nc.sync.dma_start(bk_sb[:], bk.rearrange("(hc p) -> p hc", p=P))
            bv_row = const.tile([1, HD], BF16, name="bv_row")
            nc.gpsimd.dma_start(bv_row[:], bv[None, :])
            ones_row = const.tile([1, P], BF16, name="ones_row")
            nc.vector.memset(ones_row[:], 1.0)
            onesr_f = const.tile([65, DK], F32, name="onesr_f")
            nc.vector.memset(onesr_f[64:65, :], 1.0)
            onesr = const.tile([65, DK], F32R, name="onesr")
            nc.vector.tensor_copy(onesr[64:65, :], onesr_f[64:65, :])

            m_i32 = const.tile([P, NTC], I32, name="m_i32")
            nc.sync.dma_start(m_i32[:], mask.rearrange("(o p) -> p o", p=P))
            m_f32 = const.tile([P, NTC], F32, name="m_f32")
            nc.vector.tensor_copy(m_f32[:], m_i32[:])

            khT = [kvp.tile([P, 2, TQ], BF16, name=f"khT{j}") for j in range(NQT)]
            vh = [kvp.tile([P, 4, NH_LOC * 65], BF16, name=f"vh{j}")
                  for j in range(NQT)]
            for j in range(NQT):
                for h in range(NH_LOC):
                    nc.vector.tensor_copy(vh[j][:, :, h * 65 + 64],
                                          m_f32[:, 4 * j:4 * j + 4])

            def load_transposed(src, tag):
                for qt in range(NQT):
                    xtc = xtp.tile([P, DC, TQ], BF16, name=f"xt_{tag}{qt}",
                                   tag="xt")
                    for t4 in range(4):
                        tci = qt * 4 + t4
                        nat = stage.tile([P, D], BF16, name=f"nat_{tag}{tci}",
                                         tag="nat")
                        nc.gpsimd.dma_start(nat[:], src[tci * P:(tci + 1) * P, :])
                        ptp = tpp.tile([P, DC, P], BF16, name=f"tp_{tag}{tci}",
                                       tag="tp")
                        for dc in range(DC):
                            nc.tensor.transpose(
                                ptp[:, dc, :], nat[:, dc * P:(dc + 1) * P], ident[:])
                        nc.vector.tensor_copy(
                            xtc[:, :, t4 * P:(t4 + 1) * P], ptp[:])
                    yield qt, xtc

            for qt, ktc in load_transposed(xk, "k"):
                for hc in range(2):
                    pp = ppp.tile([P, TQ], F32, name=f"ppk{qt}_{hc}", tag="pp")
                    for dc in range(DC):
                        nc.tensor.matmul(pp[:], wk_sb[:, dc, hc * P:(hc + 1) * P],
                                         ktc[:, dc, :],
                                         start=(dc == 0), stop=(dc == DC - 1))
                    nc.vector.tensor_scalar_add(khT[qt][:, hc, :], pp[:],
                                                bk_sb[:, hc:hc + 1])

            nc.gpsimd.dma_start(wv_sb[:], wv.rearrange("(dc p) h -> p dc h", p=P))
            for qt, vtc in load_transposed(xv, "v"):
                for t4 in range(4):
                    pp = ppp.tile([P, TQ], F32, name=f"ppv{qt}_{t4}", tag="pp")
                    for dc in range(DC):
                        nc.tensor.matmul(pp[:, 0:HD], vtc[:, dc, t4 * P:(t4 + 1) * P],
                                         wv_sb[:, dc, :],
                                         start=(dc == 0), stop=False)
                    nc.tensor.matmul(pp[:, 0:HD], ones_row[:], bv_row[:],
                                     start=False, stop=True)
                    tci = qt * 4 + t4
                    nc.vector.tensor_scalar_mul(
                        vh[qt][:, t4, :].rearrange("p (h x) -> p h x",
                                                   x=65)[:, :, 0:DK],
                        pp[:, 0:HD].rearrange("p (h x) -> p h x", x=DK),
                        m_f32[:, tci:tci + 1])

            nc.gpsimd.dma_start(wq_sb[:], wq.rearrange("(dc p) h -> p dc h", p=P))
            nc.gpsimd.dma_start(wo_sb[:], wo.rearrange("(hp p) n -> p hp n", p=P))
            qgen = load_transposed(xq, "q")
            for qt in range(NQT):
                _, qtc = next(qgen)
                qz = qzp.tile([P, NH_LOC, TQ], BF16, name=f"qz{qt}", tag="qz")
                for hc in range(2):
                    pp = ppp.tile([P, TQ], F32, name=f"ppq{qt}_{hc}", tag="pp")
                    for dc in range(DC):
                        nc.tensor.matmul(pp[:], wq_sb[:, dc, hc * P:(hc + 1) * P],
                                         qtc[:, dc, :],
                                         start=(dc == 0), stop=(dc == DC - 1))
                    nc.vector.tensor_scalar_add(qz[0:DK, 2 * hc, :], pp[0:DK, :],
                                                bq_sb[0:DK, hc:hc + 1])
                    nc.vector.tensor_scalar_add(qz[DK:P, 2 * hc + 1, :], pp[DK:P, :],
                                                bq_sb[DK:P, hc:hc + 1])
                    nc.vector.memset(qz[DK:P, 2 * hc, :], 0.0)
                    nc.vector.memset(qz[0:DK, 2 * hc + 1, :], 0.0)
                at_pairs = []
                for hp in range(2):
                    at_pair = smp.tile([P, TQ], BF16, name=f"atp{qt}_{hp}",
                                       tag="at")
                    for hi in range(2):
                        h = 2 * hp + hi
                        eh = etp.tile([P, NTC, TQ], BF16, name=f"e{qt}_{h}",
                                      tag="e")
                        for kc2 in range(NTC // 2):
                            ps = scp.tile([P, 2, TQ], F32, name=f"s{qt}_{h}_{kc2}",
                                          tag="s")
                            for j in range(2):
                                kc = 2 * kc2 + j
                                nc.tensor.matmul(
                                    ps[:, j, :],
                                    khT[kc // 4][:, hp, (kc % 4) * P:(kc % 4 + 1) * P],
                                    qz[:, h, :], start=True, stop=True)
                            nc.scalar.activation(eh[:, 2 * kc2:2 * kc2 + 2, :],
                                                 ps[:], Exp, scale=0.125)
                        pa = pap.tile([65, TQ], F32, name=f"pa{qt}_{h}", tag="pa")
                        for kc in range(NTC):
                            nc.tensor.matmul(
                                pa[:], vh[kc // 4][:, kc % 4, h * 65:h * 65 + 65],
                                eh[:, kc, :],
                                start=(kc == 0), stop=(kc == NTC - 1))
                        rec = smp.tile([65, TQ], F32R, name=f"rec{qt}_{h}",
                                       tag="rec")
                        with nc.allow_low_precision(reason="softmax denominator"):
                            nc.vector.reciprocal(rec[64:65, :], pa[64:65, :])
                        pb = scp.tile([P, 2, TQ], F32, name=f"pb{qt}_{h}", tag="s")
                        nc.tensor.matmul(pb[0:DK, 0, :], onesr[64:65, :],
                                         rec[64:65, :], start=True, stop=True)
                        au = smp.tile([DK, TQ], BF16, name=f"au{qt}_{h}", tag="au")
                        nc.vector.tensor_copy(au[:], pa[0:DK, :])
                        if hi == 0:
                            nc.vector.tensor_tensor(at_pair[0:DK, :], au[:],
                                                    pb[0:DK, 0, :], mult)
                        else:
                            att = smp.tile([DK, TQ], BF16, name=f"att{qt}_{h}",
                                           tag="att")
                            nc.vector.tensor_tensor(att[:], au[:],
                                                    pb[0:DK, 0, :], mult)
                            psh = scp.tile([P, 2, TQ], F32, name=f"sh{qt}_{h}",
                                           tag="s")
                            nc.tensor.matmul(psh[:, 0, :], shiftI[:], att[:],
                                             start=True, stop=True)
                            nc.vector.tensor_copy(at_pair[DK:P, :],
                                                  psh[DK:P, 0, :])
                    at_pairs.append(at_pair)

                for t4 in range(4):
                    osb = otp.tile([P, D], BF16, name=f"o{qt}_{t4}", tag="o")
                    for nh in range(2):
                        po = ppp.tile([P, TQ], F32, name=f"po{qt}_{t4}_{nh}",
                                      tag="pp")
                        for hp in range(2):
                            nc.tensor.matmul(
                                po[:], at_pairs[hp][:, t4 * P:(t4 + 1) * P],
                                wo_sb[:, hp, nh * TQ:(nh + 1) * TQ],
                                start=(hp == 0), stop=(hp == 1))
                        nc.vector.tensor_copy(osb[:, nh * TQ:(nh + 1) * TQ], po[:])
                    tci = qt * 4 + t4
                    nc.sync.dma_start(out[tci * P:(tci + 1) * P, :], osb[:])

    nc.compile()
    return nc


def _get_nc():
    global _NC
    if _NC is None:
        _NC = _build()
    return _NC


def kernel(q, k, v, mask, Wq, bq, Wk, bk, Wv, bv, Wo, bo):
    from concourse.bass_utils import run_bass_kernel_spmd

    nc = _get_nc()
    c = np.ascontiguousarray
    in_maps = []
    for core in range(8):
        b, hg = divmod(core, HG)
        s = slice(hg * HD, (hg + 1) * HD)
        in_maps.append({
            "xq": c(np.asarray(q[b], dtype=np.float32)),
            "xk": c(np.asarray(k[b], dtype=np.float32)),
            "xv": c(np.asarray(v[b], dtype=np.float32)),
            "mask": c(np.asarray(mask[b], dtype=np.int32)),
            "wq": c(np.asarray(Wq[:, s], dtype=np.float32)),
            "wk": c(np.asarray(Wk[:, s], dtype=np.float32)),
            "wv": c(np.asarray(Wv[:, s], dtype=np.float32)),
            "bq": c(np.asarray(bq[s], dtype=np.float32)),
            "bk": c(np.asarray(bk[s], dtype=np.float32)),
            "bv": c(np.asarray(bv[s], dtype=np.float32)),
            "wo": c(np.asarray(Wo[s, :], dtype=np.float32)),
        })
    res = run_bass_kernel_spmd(nc, in_maps, list(range(8)))
    outs = np.empty((N_BATCH, T, D), dtype=np.float32)
    for b in range(N_BATCH):
        acc = res.results[b * HG]["out"].astype(np.float32)
        for hg in range(1, HG):
            acc += res.results[b * HG + hg]["out"]
        outs[b] = acc + np.asarray(bo, dtype=np.float32)[None, :]
    return outs


# revision 12
# speedup vs baseline: 1.3594x; 1.0931x over previous
import numpy as np

N_BATCH = 2
T = 2048
D = 1024
HG = 4
NH_LOC = 4
DK = 64
HD = NH_LOC * DK
P = 128
TQ = 512
NTC = T // P
NQT = T // TQ
DC = D // P

_NC = None


def _build(loop_iters=None, trace_sim=False, no_dma=None):
    import os as _os
    import contextlib
    import concourse.bass as bass
    from concourse import bacc
    import concourse.mybir as mybir
    import concourse.tile as tile
    from concourse.masks import make_identity

    if no_dma is None:
        no_dma = bool(int(_os.environ.get("NODMA", "0")))

    F32 = mybir.dt.float32
    F32R = mybir.dt.float32r
    BF16 = mybir.dt.bfloat16
    I32 = mybir.dt.int32
    Exp = mybir.ActivationFunctionType.Exp
    mult = mybir.AluOpType.mult

    nc = bacc.Bacc(None, target_bir_lowering=False)
    xq = nc.dram_tensor("xq", [T, D], F32, kind="ExternalInput")
    xk = nc.dram_tensor("xk", [T, D], F32, kind="ExternalInput")
    xv = nc.dram_tensor("xv", [T, D], F32, kind="ExternalInput")
    mask = nc.dram_tensor("mask", [T], I32, kind="ExternalInput")
    wq = nc.dram_tensor("wq", [D, HD], F32, kind="ExternalInput")
    wk = nc.dram_tensor("wk", [D, HD], F32, kind="ExternalInput")
    wv = nc.dram_tensor("wv", [D, HD], F32, kind="ExternalInput")
    bq = nc.dram_tensor("bq", [HD], F32, kind="ExternalInput")
    bk = nc.dram_tensor("bk", [HD], F32, kind="ExternalInput")
    bv = nc.dram_tensor("bv", [HD], F32, kind="ExternalInput")
    wo = nc.dram_tensor("wo", [HD, D], F32, kind="ExternalInput")
    out = nc.dram_tensor("out", [T, D], BF16, kind="ExternalOutput")

    with tile.TileContext(nc, trace_sim=trace_sim) as tc:
        loop_cm = tc.For_i(0, loop_iters, 1) if loop_iters else contextlib.nullcontext()
        with loop_cm, \
             tc.tile_pool(name="const", bufs=1) as const, \
             tc.tile_pool(name="stage", bufs=8) as stage, \
             tc.tile_pool(name="xt", bufs=3) as xtp, \
             tc.tile_pool(name="kv", bufs=1) as kvp, \
             tc.tile_pool(name="qz", bufs=2) as qzp, \
             tc.tile_pool(name="et", bufs=4) as etp, \
             tc.tile_pool(name="sm", bufs=4) as smp, \
             tc.tile_pool(name="ot", bufs=2) as otp, \
             tc.tile_pool(name="tp", bufs=1, space="PSUM") as tpp, \
             tc.tile_pool(name="pp", bufs=2, space="PSUM") as ppp, \
             tc.tile_pool(name="sc", bufs=2, space="PSUM") as scp, \
             tc.tile_pool(name="pa", bufs=1, space="PSUM") as pap:

            ident = const.tile([P, P], BF16, name="ident")
            make_identity(nc, ident)
            shiftI = const.tile([DK, P], BF16, name="shiftI")
            id64 = const.tile([DK, DK], BF16, name="id64")
            make_identity(nc, id64)
            nc.vector.memset(shiftI[:, 0:DK], 0.0)
            nc.vector.tensor_copy(shiftI[:, DK:P], id64[:])

            wq_sb = const.tile([P, DC, HD], BF16, name="wq_sb")
            wk_sb = const.tile([P, DC, HD], BF16, name="wk_sb")
            wv_sb = const.tile([P, DC, HD], BF16, name="wv_sb")
            wo_sb = const.tile([P, 2, D], BF16, name="wo_sb")
            if not no_dma:
                nc.gpsimd.dma_start(wk_sb[:],
                                    wk.rearrange("(dc p) h -> p dc h", p=P))
            else:
                for t_ in (wk_sb, wv_sb, wq_sb, wo_sb):
                    nc.gpsimd.memset(t_[:], 0.01)

            bq_sb = const.tile([P, 2], F32, name="bq_sb")
            bk_sb = const.tile([P, 2], F32, name="bk_sb")
            nc.sync.dma_start(bq_sb[:], bq.rearrange("(hc p) -> p hc", p=P))
            nc.sync.dma_start(bk_sb[:], bk.rearrange("(hc p) -> p hc", p=P))
            bv_row = const.tile([1, HD], BF16, name="bv_row")
            nc.gpsimd.dma_start(bv_row[:], bv[None, :])
            ones_row = const.tile([1, P], BF16, name="ones_row")
            nc.vector.memset(ones_row[:], 1.0)
            onesr_f = const.tile([65, DK], F32, name="onesr_f")
            nc.vector.memset(onesr_f[64:65, :], 1.0)
            onesr = const.tile([65, DK], F32R, name="onesr")
            nc.vector.tensor_copy(onesr[64:65, :], onesr_f[64:65, :])

            m_i32 = const.tile([P, NTC], I32, name="m_i32")
            nc.sync.dma_start(m_i32[:], mask.rearrange("(o p) -> p o", p=P))
            m_f32 = const.tile([P, NTC], F32, name="m_f32")
            nc.vector.tensor_copy(m_f32[:], m_i32[:])

            khT = [kvp.tile([P, 2, TQ], BF16, name=f"khT{j}") for j in range(NQT)]
            vh = [kvp.tile([P, 4, NH_LOC * 65], BF16, name=f"vh{j}")
                  for j in range(NQT)]
            for j in range(NQT):
                for h in range(NH_LOC):
                    nc.vector.tensor_copy(vh[j][:, :, h * 65 + 64],
                                          m_f32[:, 4 * j:4 * j + 4])

            def load_transposed(src, tag):
                for qt in range(NQT):
                    xtc = xtp.tile([P, DC, TQ], BF16, name=f"xt_{tag}{qt}",
                                   tag="xt")
                    for t4 in range(4):
                        tci = qt * 4 + t4
                        nat = stage.tile([P, D], BF16, name=f"nat_{tag}{tci}",
                                         tag="nat")
                        if not no_dma:
                            nc.gpsimd.dma_start(nat[:],
                                                src[tci * P:(tci + 1) * P, :])
                        else:
                            nc.gpsimd.memset(nat[:], 0.25)
                        ptp = tpp.tile([P, DC, P], BF16, name=f"tp_{tag}{tci}",
                                       tag="tp")
                        for dc in range(DC):
                            nc.tensor.transpose(
                                ptp[:, dc, :], nat[:, dc * P:(dc + 1) * P],
                                ident[:])
                        nc.vector.tensor_copy(
                            xtc[:, :, t4 * P:(t4 + 1) * P], ptp[:])
                    yield qt, xtc

            for qt, ktc in load_transposed(xk, "k"):
                for hc in range(2):
                    pp = ppp.tile([P, TQ], F32, name=f"ppk{qt}_{hc}", tag="pp")
                    for dc in range(DC):
                        nc.tensor.matmul(pp[:], wk_sb[:, dc, hc * P:(hc + 1) * P],
                                         ktc[:, dc, :],
                                         start=(dc == 0), stop=(dc == DC - 1))
                    nc.vector.tensor_scalar_add(khT[qt][:, hc, :], pp[:],
                                                bk_sb[:, hc:hc + 1])

            if not no_dma:
                nc.gpsimd.dma_start(wv_sb[:],
                                    wv.rearrange("(dc p) h -> p dc h", p=P))
            for qt, vtc in load_transposed(xv, "v"):
                for t4 in range(4):
                    pp = ppp.tile([P, TQ], F32, name=f"ppv{qt}_{t4}", tag="pp")
                    for dc in range(DC):
                        nc.tensor.matmul(pp[:, 0:HD],
                                         vtc[:, dc, t4 * P:(t4 + 1) * P],
                                         wv_sb[:, dc, :],
                                         start=(dc == 0), stop=False)
                    nc.tensor.matmul(pp[:, 0:HD], ones_row[:], bv_row[:],
                                     start=False, stop=True)
                    tci = qt * 4 + t4
                    nc.vector.tensor_scalar_mul(
                        vh[qt][:, t4, :].rearrange("p (h x) -> p h x",
                                                   x=65)[:, :, 0:DK],
                        pp[:, 0:HD].rearrange("p (h x) -> p h x", x=DK),
                        m_f32[:, tci:tci + 1])

            if not no_dma:
                nc.gpsimd.dma_start(wq_sb[:],
                                    wq.rearrange("(dc p) h -> p dc h", p=P))
                nc.gpsimd.dma_start(wo_sb[:],
                                    wo.rearrange("(hp p) n -> p hp n", p=P))

            qgen = load_transposed(xq, "q")
            for qt in range(NQT):
                _, qtc = next(qgen)
                qz = qzp.tile([P, NH_LOC, TQ], BF16, name=f"qz{qt}", tag="qz")
                for hc in range(2):
                    pp = ppp.tile([P, TQ], F32, name=f"ppq{qt}_{hc}", tag="pp")
                    for dc in range(DC):
                        nc.tensor.matmul(pp[:], wq_sb[:, dc, hc * P:(hc + 1) * P],
                                         qtc[:, dc, :],
                                         start=(dc == 0), stop=(dc == DC - 1))
                    nc.vector.tensor_scalar_add(qz[0:DK, 2 * hc, :], pp[0:DK, :],
                                                bq_sb[0:DK, hc:hc + 1])
                    nc.vector.tensor_scalar_add(qz[DK:P, 2 * hc + 1, :],
                                                pp[DK:P, :],
                                                bq_sb[DK:P, hc:hc + 1])
                    nc.vector.memset(qz[DK:P, 2 * hc, :], 0.0)
                    nc.vector.memset(qz[0:DK, 2 * hc + 1, :], 0.0)

                ehs = {}
                pas = {}
                at_pairs = []
                at_pair = None

                def emit_score_pair(h, kc2):
                    hp = h // 2
                    ps = scp.tile([P, 2, TQ], F32, name=f"s{qt}_{h}_{kc2}",
                                  tag="s")
                    for j in range(2):
                        kc = 2 * kc2 + j
                        nc.tensor.matmul(
                            ps[:, j, :],
                            khT[kc // 4][:, hp, (kc % 4) * P:(kc % 4 + 1) * P],
                            qz[:, h, :], start=True, stop=True)
                    nc.scalar.activation(ehs[h][:, 2 * kc2:2 * kc2 + 2, :],
                                         ps[:], Exp, scale=0.125)

                def emit_pv_chunks(h, kcs):
                    for kc in kcs:
                        nc.tensor.matmul(
                            pas[h][:], vh[kc // 4][:, kc % 4, h * 65:h * 65 + 65],
                            ehs[h][:, kc, :],
                            start=(kc == 0), stop=(kc == NTC - 1))

                def emit_norm(h):
                    pa = pas[h]
                    rec = smp.tile([65, TQ], F32R, name=f"rec{qt}_{h}", tag="rec")
                    with nc.allow_low_precision(reason="softmax denominator"):
                        nc.vector.reciprocal(rec[64:65, :], pa[64:65, :])
                    pb = scp.tile([P, 2, TQ], F32, name=f"pb{qt}_{h}", tag="s")
                    nc.tensor.matmul(pb[0:DK, 0, :], onesr[64:65, :],
                                     rec[64:65, :], start=True, stop=True)
                    au = smp.tile([DK, TQ], BF16, name=f"au{qt}_{h}", tag="au")
                    nc.vector.tensor_copy(au[:], pa[0:DK, :])
                    if h % 2 == 0:
                        nc.vector.tensor_tensor(at_pair[0:DK, :], au[:],
                                                pb[0:DK, 0, :], mult)
                    else:
                        att = smp.tile([DK, TQ], BF16, name=f"att{qt}_{h}",
                                       tag="att")
                        nc.vector.tensor_tensor(att[:], au[:], pb[0:DK, 0, :],
                                                mult)
                        psh = scp.tile([P, 2, TQ], F32, name=f"sh{qt}_{h}",
                                       tag="s")
                        nc.tensor.matmul(psh[:, 0, :], shiftI[:], att[:],
                                         start=True, stop=True)
                        nc.vector.tensor_copy(at_pair[DK:P, :], psh[DK:P, 0, :])

                ehs[0] = etp.tile([P, NTC, TQ], BF16, name=f"e{qt}_0", tag="e")
                for kc2 in range(NTC // 2):
                    emit_score_pair(0, kc2)
                for h in range(NH_LOC):
                    if h % 2 == 0:
                        at_pair = smp.tile([P, TQ], BF16,
                                           name=f"atp{qt}_{h // 2}", tag="at")
                        at_pairs.append(at_pair)
                    pas[h] = pap.tile([65, TQ], F32, name=f"pa{qt}_{h}",
                                      tag="pa")
                    hn = h + 1
                    if hn < NH_LOC:
                        ehs[hn] = etp.tile([P, NTC, TQ], BF16,
                                           name=f"e{qt}_{hn}", tag="e")
                        for kc2 in range(NTC // 2):
                            emit_score_pair(hn, kc2)
                            emit_pv_chunks(h, [2 * kc2, 2 * kc2 + 1])
                    else:
                        emit_pv_chunks(h, list(range(NTC)))
                    emit_norm(h)

                for t4 in range(4):
                    osb = otp.tile([P, D], BF16, name=f"o{qt}_{t4}", tag="o")
                    for nh in range(2):
                        po = ppp.tile([P, TQ], F32, name=f"po{qt}_{t4}_{nh}",
                                      tag="pp")
                        for hp in range(2):
                            nc.tensor.matmul(
                                po[:], at_pairs[hp][:, t4 * P:(t4 + 1) * P],
                                wo_sb[:, hp, nh * TQ:(nh + 1) * TQ],
                                start=(hp == 0), stop=(hp == 1))
                        nc.vector.tensor_copy(osb[:, nh * TQ:(nh + 1) * TQ],
                                              po[:])
                    tci = qt * 4 + t4
                    nc.sync.dma_start(out[tci * P:(tci + 1) * P, :], osb[:])

    nc.compile()
    return nc


def _get_nc():
    global _NC
    if _NC is None:
        _NC = _build()
    return _NC


def kernel(q, k, v, mask, Wq, bq, Wk, bk, Wv, bv, Wo, bo):
    from concourse.bass_utils import run_bass_kernel_spmd

    nc = _get_nc()
    c = np.ascontiguousarray
    in_maps = []
    for core in range(8):
        b, hg = divmod(core, HG)
        s = slice(hg * HD, (hg + 1) * HD)
        in_maps.append({
            "xq": c(np.asarray(q[b], dtype=np.float32)),
            "xk": c(np.asarray(k[b], dtype=np.float32)),
            "xv": c(np.asarray(v[b], dtype=np.float32)),
            "mask": c(np.asarray(mask[b], dtype=np.int32)),
            "wq": c(np.asarray(Wq[:, s], dtype=np.float32)),
            "wk": c(np.asarray(Wk[:, s], dtype=np.float32)),
            "wv": c(np.asarray(Wv[:, s], dtype=np.float32)),
            "bq": c(np.asarray(bq[s], dtype=np.float32)),
            "bk": c(np.asarray(bk[s], dtype=np.float32)),
            "bv": c(np.asarray(bv[s], dtype=np.float32)),
            "wo": c(np.asarray(Wo[s, :], dtype=np.float32)),
        })
    res = run_bass_kernel_spmd(nc, in_maps, list(range(8)))
    outs = np.empty((N_BATCH, T, D), dtype=np.float32)
    for b in range(N_BATCH):
        acc = res.results[b * HG]["out"].astype(np.float32)
        for hg in range(1, HG):
            acc += res.results[b * HG + hg]["out"].astype(np.float32)
        outs[b] = acc + np.asarray(bo, dtype=np.float32)[None, :]
    return outs
